# revision 22
# baseline (speedup 1.0000x reference)
"""Trainium2 Bass kernel for CustomMultiHeadAttention (single-query pooled attention).

Reference computation (B=32, S=1024, D=256, H=8):
    keys   = (x @ Wk + bk).reshape(B,S,H,D)
    values = (x @ Wv + bv).reshape(B,S,H,D)
    scores = einsum('bshd,hd->bsh', keys, query)
    attn   = softmax(scores, axis=1)           # over S
    pooled = einsum('bsh,bshd->bhd', attn, values).reshape(B, H*D)
    out    = pooled @ Wo + bo

Algebraic restructure (exact in real arithmetic):
    qp[e,h]  = sum_d Wk[e, h*D+d] * query[h,d]          # [256, 8]   (host fold)
    scores[b,s,h] = x[b,s,:] @ qp[:,h]   (+ const(h) from bk -> cancels in softmax)
    attnu = exp(scores - 64)                            # const shift; softmax invariant
    ctx[b,h,e] = sum_s attnu[b,s,h] * x[b,s,e];  Z[b,h] = sum_s attnu[b,s,h]
    Wvo[h]   = Wv_h @ Wo_h                              # [256, 256] per head (host fold)
    out[b]   = sum_h (ctx[b,h,:]/Z[b,h]) @ Wvo[h] + (bv @ Wo + bo)   # bias on host

Device mapping (all matmuls bf16 with fp32 PSUM accumulation):
  - Every load is a bf16 DMA-crossbar transpose on one queue: the tile
    framework chains DMA completions in tick order and only exempts
    consecutive same-type transfers, so a single homogeneous run is the only
    layout-conversion scheme with zero chain stalls. The host pre-transposes
    qp/Wvo/x so each SBUF destination layout is one transpose away.
  - x lands twice: transposed [e, s] (scores operand, from natural-layout
    DRAM) and natural [s, e] (ctx operand, from host-transposed DRAM),
    interleaved per batch so compute starts after the first batch arrives.
  - scoresT[h, s] = qp_k.T @ xT streams 512-wide; exp runs on the Activation
    engine with fused accum_out giving Z = sum_s attn for free.
  - attn comes back to [s, (b,h)] via one PE transpose per (s-chunk, batch
    pair); ctx for a batch pair is one matmul chain attn_chunk.T [16] @
    [x_b0 | x_b1] (the off-diagonal half of the products is discarded).
  - out = sum_kh ctxT.T @ Wvo with PSUM accumulation; bias applied on host.

Sharding: data-parallel over batch, 4 batches per core on 8 cores.
"""

import sys

sys.path.insert(0, "/opt/trn_rl_repo")

import numpy as np
import ml_dtypes

import concourse.bass as bass
import concourse.mybir as mybir
import concourse.tile as tile
from concourse import bacc
from concourse.bass_utils import run_bass_kernel_spmd
from concourse.masks import make_identity

F32 = mybir.dt.float32
F16 = mybir.dt.float16
F16_NP = np.float16

B, S, D, H = 32, 1024, 256, 8
NCORES = 8
BL = B // NCORES      # local batches per core = 4
ST = S // 128         # s-tiles per batch = 8
KD = 2                # 256 = 2 k-tiles of 128 over the e (input dim) axis
NP_ = 2               # batch pairs per core


def build_program(reps: int = 1):
    nc = bacc.Bacc("TRN2", target_bir_lowering=False, debug=False)

    xs_d = nc.dram_tensor("xs", [BL, S, D], F16, kind="ExternalInput")
    xtd_d = nc.dram_tensor("xtd", [BL, D, S], F16, kind="ExternalInput")
    qp_d = nc.dram_tensor("qp", [D, 16], F16, kind="ExternalInput")
    wvo_d = nc.dram_tensor("wvo", [D, H, D], F16, kind="ExternalInput")
    out_d = nc.dram_tensor("out", [BL, D], F32, kind="ExternalOutput")

    with tile.TileContext(nc) as tc:
        with (
            tc.tile_pool(name="big", bufs=1) as big,
            tc.tile_pool(name="sm", bufs=1) as sm,
            tc.tile_pool(name="ps", bufs=1, space=bass.MemorySpace.PSUM) as ps,
            tc.tile_pool(name="pst", bufs=2, space=bass.MemorySpace.PSUM) as pst,
        ):
            # ---- SBUF allocations (2 phases for cross-iteration ------
            # ---- pipelining in the timing loop) ----------------------
            NPH = 2
            xn_sb = [big.tile([128, BL, ST, D], F16, name=f"xn{p}") for p in range(NPH)]
            xt_sb = [big.tile([128, KD, BL, S], F16, name=f"xt{p}") for p in range(NPH)]
            qp_sb = [sm.tile([128, KD, 16], F16, name=f"qp{p}") for p in range(NPH)]
            wvo_sb = [big.tile([128, KD, H, D], F16, name=f"wvo{p}") for p in range(NPH)]
            attnT_sb = [sm.tile([128, S], F16, name=f"aT{p}") for p in range(NPH)]
            attn_sb = [sm.tile([128, ST, NP_, 40], F16, name=f"at{p}") for p in range(NPH)]
            zsum = [sm.tile([H, BL, 2], F32, name=f"zs{p}") for p in range(NPH)]
            zt = [sm.tile([H, BL], F32, name=f"zt{p}") for p in range(NPH)]
            recip = [sm.tile([H, BL], F32, name=f"rc{p}") for p in range(NPH)]
            ctxn_sb = [sm.tile([H, BL, D], F32, name=f"cn{p}") for p in range(NPH)]
            ctxT_sb = [sm.tile([128, KD, BL, H], F16, name=f"cT{p}") for p in range(NPH)]
            ident_bf = sm.tile([128, 40], F16)           # I40 at rows 0:40 and 64:104
            ident8 = sm.tile([8, 8], F32)
            mh = [sm.tile([H, BL, 2], F32, name=f"mh{p}") for p in range(NPH)]
            mm = [sm.tile([H, BL], F32, name=f"mmx{p}") for p in range(NPH)]
            negm = [sm.tile([H, BL], F32, name=f"nm{p}") for p in range(NPH)]
            out_sb = [sm.tile([BL, D], F32, name=f"ou{p}") for p in range(NPH)]

            def body(ph):
                # ---- one homogeneous run of plain rearrange copies ----
                # (the host ships x in both layouts, so no device-side
                # transposes are needed and the DMA chain never stalls)
                nc.sync.dma_start(
                    qp_sb[ph][:], qp_d[:].rearrange("(k p) h -> p k h", p=128)
                )
                for b in range(BL):
                    nc.sync.dma_start(
                        xt_sb[ph][:, :, b, :],
                        xtd_d[b].rearrange("(k p) s -> p k s", p=128),
                    )
                    nc.sync.dma_start(
                        xn_sb[ph][:, b, :, :],
                        xs_d[b].rearrange("(t p) e -> p t e", p=128),
                    )
                for k in range(KD):
                    nc.sync.dma_start(
                        wvo_sb[ph][:, k],
                        wvo_d[k * 128:(k + 1) * 128].rearrange("p h d -> p h d"),
                    )


                # ---- scoresT[h, s] = qp_k.T @ xT, exp on ACT ---------
                # k-major pairs keep the qp_k stationary loaded across both
                # s-halves (one ldweights per (b, k))
                for b in range(BL):
                    sc = [pst.tile([8, 512], F32, tag="sc", name=f"sc{b}_{j}")
                          for j in range(2)]
                    for k in range(KD):
                        for half in range(2):
                            nc.tensor.matmul(
                                sc[half][:],
                                qp_sb[ph][:, k, 0:8],
                                xt_sb[ph][:, k, b, half * 512:(half + 1) * 512],
                                start=(k == 0),
                                stop=(k == KD - 1),
                            )
                    # stable softmax: subtract the per-(b,h) max so exp
                    # outputs live in (0, 1] and fit fp16
                    for half in range(2):
                        nc.vector.reduce_max(
                            mh[ph][:, b, half:half + 1], sc[half][:],
                            axis=mybir.AxisListType.X,
                        )
                    nc.vector.tensor_max(
                        mm[ph][:, b:b + 1], mh[ph][:, b, 0:1], mh[ph][:, b, 1:2]
                    )
                    nc.vector.tensor_scalar_mul(
                        negm[ph][:, b:b + 1], mm[ph][:, b:b + 1], -1.0
                    )
                    row = 32 * b
                    for half in range(2):
                        nc.scalar.activation(
                            attnT_sb[ph][row:row + 8, half * 512:(half + 1) * 512],
                            sc[half][:],
                            mybir.ActivationFunctionType.Exp,
                            bias=negm[ph][:, b:b + 1],
                            accum_out=zsum[ph][:, b, half:half + 1],
                        )

                # ---- attn[s, pair 16] via PE transpose per (chunk, pair)
                for i in range(NP_):
                    for t in range(ST):
                        atp = pst.tile([128, 40], F16, tag="tp")
                        nc.tensor.transpose(
                            atp[:],
                            attnT_sb[ph][64 * i:64 * i + 40, t * 128:(t + 1) * 128],
                            ident_bf[64 * i:64 * i + 40, :],
                        )
                        nc.vector.tensor_copy(attn_sb[ph][:, t, i, :], atp[:])

                # 1/Z for all (h, b) in two DVE ops
                nc.vector.tensor_add(zt[ph][:], zsum[ph][:, :, 0], zsum[ph][:, :, 1])
                nc.vector.reciprocal(recip[ph][:], zt[ph][:])

                # ---- ctx per batch pair: attn_chunk.T @ [x_b0 | x_b1] -
                # ctxw[i][(b,h), j, e] = sum_s attn[s,(b,h)] * xn[2i+j][s,e];
                # only the j-th 8-row band of block column j is read back.
                ctxw = [
                    ps.tile([40, 2, D], F32, tag=f"cw{i}", name=f"cw{i}")
                    for i in range(NP_)
                ]
                for t in range(ST):
                    for i in range(NP_):
                        nc.tensor.matmul(
                            ctxw[i][:],
                            attn_sb[ph][:, t, i, :],
                            xn_sb[ph][:, 2 * i:2 * i + 2, t, :],
                            start=(t == 0),
                            stop=(t == ST - 1),
                        )
                for b in range(BL):
                    i, j = b // 2, b % 2
                    nc.vector.tensor_scalar_mul(
                        ctxn_sb[ph][:, b, :],
                        ctxw[i][32 * j:32 * j + 8, j, :],
                        recip[ph][:, b:b + 1],
                    )
                    for k in range(KD):
                        ctp = pst.tile([128, H], F32, tag="tp")
                        nc.tensor.transpose(
                            ctp[:],
                            ctxn_sb[ph][:, b, k * 128:(k + 1) * 128],
                            ident8[:],
                        )
                        nc.vector.tensor_copy(ctxT_sb[ph][:, k, b, :], ctp[:])

                # ---- out[b, :] = sum_{k,h} ctxT_kh.T @ Wvo_kh --------
                out_ps = ps.tile([BL, D], F32, tag=f"fin{ph}", name=f"fin{ph}")
                for k in range(KD):
                    for h in range(H):
                        nc.tensor.matmul(
                            out_ps[:],
                            ctxT_sb[ph][:, k, :, h],
                            wvo_sb[ph][:, k, h, :],
                            start=(k == 0 and h == 0),
                            stop=(k == KD - 1 and h == H - 1),
                        )
                nc.vector.tensor_copy(out_sb[ph][:], out_ps[:])
                nc.sync.dma_start(out_d[:], out_sb[ph][:])

            make_identity(nc, ident_bf[0:40, :])
            make_identity(nc, ident_bf[64:104, :])
            make_identity(nc, ident8[:])
            # zero the junk rows inside each pair's 40-row slab once: exp only
            # ever writes the live rows, so the discarded ctx rows stay finite
            # across iterations (32-aligned bands cover the junk)
            for p in range(NPH):
                nc.vector.memset(attnT_sb[p][0:32, :], 0.0)
                nc.vector.memset(attnT_sb[p][64:96, :], 0.0)
            if reps == 1:
                body(0)
            else:
                assert reps % 2 == 1
                body(0)
                with tc.For_i(0, (reps - 1) // 2):
                    body(1)
                    body(0)

    nc.compile()
    return nc


_NC_CACHE = {}


def get_nc(reps: int = 1):
    if reps not in _NC_CACHE:
        _NC_CACHE[reps] = build_program(reps)
    return _NC_CACHE[reps]


def make_in_maps(x, Wk, bk, Wv, bv, query, Wo, bo):
    x = np.asarray(x, dtype=np.float32)
    Wk = np.asarray(Wk, dtype=np.float32)
    Wv = np.asarray(Wv, dtype=np.float32)
    Wo = np.asarray(Wo, dtype=np.float32)
    query = np.asarray(query, dtype=np.float32)
    bv = np.asarray(bv, dtype=np.float32)
    bo = np.asarray(bo, dtype=np.float32)

    # host weight folds (weights-only; in deployment these are offline consts)
    qp = np.einsum("ehd,hd->eh", Wk.reshape(D, H, D), query)          # [256, 8]
    wvo = np.matmul(
        Wv.reshape(D, H, D).transpose(1, 0, 2),                       # [h, e, d]
        Wo.reshape(H, D, D),                                          # [h, d, f]
    )                                                                 # [h, e, f]
    bias_total = bv @ Wo + bo                                         # [256]

    xbf = np.ascontiguousarray(x.astype(F16_NP))
    xtd = np.ascontiguousarray(x.transpose(0, 2, 1).astype(F16_NP))  # [B, D, S]
    qpn = np.zeros((D, 16), dtype=F16_NP)
    qpn[:, :H] = qp.astype(F16_NP)                                   # [256, 16]
    wvon = np.ascontiguousarray(wvo.transpose(1, 0, 2)).astype(F16_NP)  # [e,h,f]

    in_maps = []
    for c in range(NCORES):
        in_maps.append(
            {
                "xs": xbf[c * BL:(c + 1) * BL],
                "xtd": xtd[c * BL:(c + 1) * BL],
                "qp": qpn,
                "wvo": wvon,
            }
        )
    return in_maps, bias_total


def kernel(x, Wk, bk, Wv, bv, query, Wo, bo):
    nc = get_nc()
    in_maps, bias_total = make_in_maps(x, Wk, bk, Wv, bv, query, Wo, bo)
    res = run_bass_kernel_spmd(nc, in_maps, core_ids=list(range(NCORES)))
    out = np.concatenate([res.results[c]["out"] for c in range(NCORES)], axis=0)
    return (out + bias_total[None, :]).astype(np.float32)


# revision 25
# speedup vs baseline: 1.2830x; 1.2830x over previous
"""Trainium2 Bass kernel for CustomMultiHeadAttention (single-query pooled attention).

Reference computation (B=32, S=1024, D=256, H=8):
    keys   = (x @ Wk + bk).reshape(B,S,H,D)
    values = (x @ Wv + bv).reshape(B,S,H,D)
    scores = einsum('bshd,hd->bsh', keys, query)
    attn   = softmax(scores, axis=1)           # over S
    pooled = einsum('bsh,bshd->bhd', attn, values).reshape(B, H*D)
    out    = pooled @ Wo + bo

Algebraic restructure (exact in real arithmetic):
    qp[e,h]  = sum_d Wk[e, h*D+d] * query[h,d]          # [256, 8]   (host fold)
    scores[b,s,h] = x[b,s,:] @ qp[:,h]   (+ const(h) from bk -> cancels in softmax)
    attnu = exp(scores - 64)                            # const shift; softmax invariant
    ctx[b,h,e] = sum_s attnu[b,s,h] * x[b,s,e];  Z[b,h] = sum_s attnu[b,s,h]
    Wvo[h]   = Wv_h @ Wo_h                              # [256, 256] per head (host fold)
    out[b]   = sum_h (ctx[b,h,:]/Z[b,h]) @ Wvo[h] + (bv @ Wo + bo)   # bias on host

Device mapping (all matmuls bf16 with fp32 PSUM accumulation):
  - Every load is a bf16 DMA-crossbar transpose on one queue: the tile
    framework chains DMA completions in tick order and only exempts
    consecutive same-type transfers, so a single homogeneous run is the only
    layout-conversion scheme with zero chain stalls. The host pre-transposes
    qp/Wvo/x so each SBUF destination layout is one transpose away.
  - x lands twice: transposed [e, s] (scores operand, from natural-layout
    DRAM) and natural [s, e] (ctx operand, from host-transposed DRAM),
    interleaved per batch so compute starts after the first batch arrives.
  - scoresT[h, s] = qp_k.T @ xT streams 512-wide; exp runs on the Activation
    engine with fused accum_out giving Z = sum_s attn for free.
  - attn comes back to [s, (b,h)] via one PE transpose per (s-chunk, batch
    pair); ctx for a batch pair is one matmul chain attn_chunk.T [16] @
    [x_b0 | x_b1] (the off-diagonal half of the products is discarded).
  - out = sum_kh ctxT.T @ Wvo with PSUM accumulation; bias applied on host.

Sharding: data-parallel over batch, 4 batches per core on 8 cores.
"""

import sys

sys.path.insert(0, "/opt/trn_rl_repo")

import numpy as np
import ml_dtypes

import concourse.bass as bass
import concourse.mybir as mybir
import concourse.tile as tile
from concourse import bacc
from concourse.bass_utils import run_bass_kernel_spmd
from concourse.masks import make_identity

F32 = mybir.dt.float32
F16 = mybir.dt.float16
F16_NP = np.float16

B, S, D, H = 32, 1024, 256, 8
NCORES = 8
BL = B // NCORES      # local batches per core = 4
ST = S // 128         # s-tiles per batch = 8
KD = 2                # 256 = 2 k-tiles of 128 over the e (input dim) axis
NP_ = 2               # batch pairs per core


def build_program(reps: int = 1):
    nc = bacc.Bacc("TRN2", target_bir_lowering=False, debug=False)

    xs_d = nc.dram_tensor("xs", [BL, S, D], F16, kind="ExternalInput")
    xtd_d = nc.dram_tensor("xtd", [BL, D, S], F16, kind="ExternalInput")
    qp_d = nc.dram_tensor("qp", [D, 16], F16, kind="ExternalInput")
    wvo_d = nc.dram_tensor("wvo", [D, H, D], F16, kind="ExternalInput")
    out_d = nc.dram_tensor("out", [BL, D], F32, kind="ExternalOutput")

    with tile.TileContext(nc) as tc:
        with (
            tc.tile_pool(name="big", bufs=1) as big,
            tc.tile_pool(name="sm", bufs=1) as sm,
            tc.tile_pool(name="ps", bufs=1, space=bass.MemorySpace.PSUM) as ps,
            tc.tile_pool(name="pst", bufs=2, space=bass.MemorySpace.PSUM) as pst,
        ):
            # ---- SBUF allocations (2 phases for cross-iteration ------
            # ---- pipelining in the timing loop) ----------------------
            NPH = 2
            xn_sb = [big.tile([128, BL, ST, D], F16, name=f"xn{p}") for p in range(NPH)]
            xt_sb = [big.tile([128, KD, BL, S], F16, name=f"xt{p}") for p in range(NPH)]
            qp_sb = [sm.tile([128, KD, 16], F16, name=f"qp{p}") for p in range(NPH)]
            wvo_sb = [big.tile([128, KD, H, D], F16, name=f"wvo{p}") for p in range(NPH)]
            attnT_sb = [sm.tile([128, S], F16, name=f"aT{p}") for p in range(NPH)]
            attn_sb = [sm.tile([128, ST, NP_, 40], F16, name=f"at{p}") for p in range(NPH)]
            zsum = [sm.tile([H, BL, 2], F32, name=f"zs{p}") for p in range(NPH)]
            zt = [sm.tile([H, BL], F32, name=f"zt{p}") for p in range(NPH)]
            recip = [sm.tile([H, BL], F32, name=f"rc{p}") for p in range(NPH)]
            ctxn_sb = [sm.tile([H, BL, D], F32, name=f"cn{p}") for p in range(NPH)]
            ctxT_sb = [sm.tile([128, KD, BL, H], F16, name=f"cT{p}") for p in range(NPH)]
            ident_bf = sm.tile([128, 40], F16)           # I40 at rows 0:40 and 64:104
            ident8 = sm.tile([8, 8], F32)
            mh = [sm.tile([H, BL, 2], F32, name=f"mh{p}") for p in range(NPH)]
            mm = [sm.tile([H, BL], F32, name=f"mmx{p}") for p in range(NPH)]
            negm = [sm.tile([H, BL], F32, name=f"nm{p}") for p in range(NPH)]
            out_sb = [sm.tile([BL, D], F32, name=f"ou{p}") for p in range(NPH)]

            def body(ph):
                # ---- one homogeneous run of plain rearrange copies ----
                # (the host ships x in both layouts, so no device-side
                # transposes are needed and the DMA chain never stalls)
                nc.sync.dma_start(
                    qp_sb[ph][:], qp_d[:].rearrange("(k p) h -> p k h", p=128)
                )
                for b in range(BL):
                    nc.sync.dma_start(
                        xt_sb[ph][:, :, b, :],
                        xtd_d[b].rearrange("(k p) s -> p k s", p=128),
                    )
                    nc.sync.dma_start(
                        xn_sb[ph][:, b, :, :],
                        xs_d[b].rearrange("(t p) e -> p t e", p=128),
                    )
                for k in range(KD):
                    nc.sync.dma_start(
                        wvo_sb[ph][:, k],
                        wvo_d[k * 128:(k + 1) * 128].rearrange("p h d -> p h d"),
                    )

                # junk rows inside each pair's 40-row slab must stay finite
                # for the discarded ctx rows (32-aligned bands cover them)
                nc.vector.memset(attnT_sb[ph][0:32, :], 0.0)
                nc.vector.memset(attnT_sb[ph][64:96, :], 0.0)

                # ---- scoresT[h, s] = qp_k.T @ xT, exp on ACT ---------
                # k-major pairs keep the qp_k stationary loaded across both
                # s-halves (one ldweights per (b, k))
                for b in range(BL):
                    sc = [pst.tile([8, 512], F32, tag="sc", name=f"sc{b}_{j}", bufs=3)
                          for j in range(2)]
                    for k in range(KD):
                        for half in range(2):
                            nc.tensor.matmul(
                                sc[half][:],
                                qp_sb[ph][:, k, 0:8],
                                xt_sb[ph][:, k, b, half * 512:(half + 1) * 512],
                                start=(k == 0),
                                stop=(k == KD - 1),
                            )
                    # stable softmax: subtract the per-(b,h) max so exp
                    # outputs live in (0, 1] and fit fp16
                    for half in range(2):
                        nc.vector.reduce_max(
                            mh[ph][:, b, half:half + 1], sc[half][:],
                            axis=mybir.AxisListType.X,
                        )
                    nc.vector.tensor_max(
                        mm[ph][:, b:b + 1], mh[ph][:, b, 0:1], mh[ph][:, b, 1:2]
                    )
                    nc.vector.tensor_scalar_mul(
                        negm[ph][:, b:b + 1], mm[ph][:, b:b + 1], -1.0
                    )
                    row = 32 * b
                    for half in range(2):
                        nc.scalar.activation(
                            attnT_sb[ph][row:row + 8, half * 512:(half + 1) * 512],
                            sc[half][:],
                            mybir.ActivationFunctionType.Exp,
                            bias=negm[ph][:, b:b + 1],
                            accum_out=zsum[ph][:, b, half:half + 1],
                        )

                # ---- attn[s, pair 16] via PE transpose per (chunk, pair)
                for i in range(NP_):
                    for t in range(ST):
                        atp = pst.tile([128, 40], F16, tag="tp")
                        nc.tensor.transpose(
                            atp[:],
                            attnT_sb[ph][64 * i:64 * i + 40, t * 128:(t + 1) * 128],
                            ident_bf[64 * i:64 * i + 40, :],
                        )
                        nc.vector.tensor_copy(attn_sb[ph][:, t, i, :], atp[:])

                # 1/Z for all (h, b) in two DVE ops
                nc.vector.tensor_add(zt[ph][:], zsum[ph][:, :, 0], zsum[ph][:, :, 1])
                nc.vector.reciprocal(recip[ph][:], zt[ph][:])

                # ---- ctx per batch pair: attn_chunk.T @ [x_b0 | x_b1] -
                # ctxw[i][(b,h), j, e] = sum_s attn[s,(b,h)] * xn[2i+j][s,e];
                # only the j-th 8-row band of block column j is read back.
                ctxw = [
                    ps.tile([40, 2, D], F32, tag=f"cw{i}", name=f"cw{i}")
                    for i in range(NP_)
                ]
                for t in range(ST):
                    for i in range(NP_):
                        nc.tensor.matmul(
                            ctxw[i][:],
                            attn_sb[ph][:, t, i, :],
                            xn_sb[ph][:, 2 * i:2 * i + 2, t, :],
                            start=(t == 0),
                            stop=(t == ST - 1),
                        )
                for b in range(BL):
                    i, j = b // 2, b % 2
                    nc.vector.tensor_scalar_mul(
                        ctxn_sb[ph][:, b, :],
                        ctxw[i][32 * j:32 * j + 8, j, :],
                        recip[ph][:, b:b + 1],
                    )
                    for k in range(KD):
                        ctp = pst.tile([128, H], F32, tag="tp")
                        nc.tensor.transpose(
                            ctp[:],
                            ctxn_sb[ph][:, b, k * 128:(k + 1) * 128],
                            ident8[:],
                        )
                        nc.vector.tensor_copy(ctxT_sb[ph][:, k, b, :], ctp[:])

                # ---- out[b, :] = sum_{k,h} ctxT_kh.T @ Wvo_kh --------
                out_ps = ps.tile([BL, D], F32, tag="fin")
                for k in range(KD):
                    for h in range(H):
                        nc.tensor.matmul(
                            out_ps[:],
                            ctxT_sb[ph][:, k, :, h],
                            wvo_sb[ph][:, k, h, :],
                            start=(k == 0 and h == 0),
                            stop=(k == KD - 1 and h == H - 1),
                        )
                nc.vector.tensor_copy(out_sb[ph][:], out_ps[:])
                nc.sync.dma_start(out_d[:], out_sb[ph][:])

            make_identity(nc, ident_bf[0:40, :])
            make_identity(nc, ident_bf[64:104, :])
            make_identity(nc, ident8[:])
            if reps == 1:
                body(0)
            else:
                assert reps % 2 == 1
                body(0)
                with tc.For_i(0, (reps - 1) // 2):
                    body(1)
                    body(0)

    nc.compile()
    return nc


_NC_CACHE = {}


def get_nc(reps: int = 1):
    if reps not in _NC_CACHE:
        _NC_CACHE[reps] = build_program(reps)
    return _NC_CACHE[reps]


def make_in_maps(x, Wk, bk, Wv, bv, query, Wo, bo):
    x = np.asarray(x, dtype=np.float32)
    Wk = np.asarray(Wk, dtype=np.float32)
    Wv = np.asarray(Wv, dtype=np.float32)
    Wo = np.asarray(Wo, dtype=np.float32)
    query = np.asarray(query, dtype=np.float32)
    bv = np.asarray(bv, dtype=np.float32)
    bo = np.asarray(bo, dtype=np.float32)

    # host weight folds (weights-only; in deployment these are offline consts)
    qp = np.einsum("ehd,hd->eh", Wk.reshape(D, H, D), query)          # [256, 8]
    wvo = np.matmul(
        Wv.reshape(D, H, D).transpose(1, 0, 2),                       # [h, e, d]
        Wo.reshape(H, D, D),                                          # [h, d, f]
    )                                                                 # [h, e, f]
    bias_total = bv @ Wo + bo                                         # [256]

    xbf = np.ascontiguousarray(x.astype(F16_NP))
    xtd = np.ascontiguousarray(x.transpose(0, 2, 1).astype(F16_NP))  # [B, D, S]
    qpn = np.zeros((D, 16), dtype=F16_NP)
    qpn[:, :H] = qp.astype(F16_NP)                                   # [256, 16]
    wvon = np.ascontiguousarray(wvo.transpose(1, 0, 2)).astype(F16_NP)  # [e,h,f]

    in_maps = []
    for c in range(NCORES):
        in_maps.append(
            {
                "xs": xbf[c * BL:(c + 1) * BL],
                "xtd": xtd[c * BL:(c + 1) * BL],
                "qp": qpn,
                "wvo": wvon,
            }
        )
    return in_maps, bias_total


def kernel(x, Wk, bk, Wv, bv, query, Wo, bo):
    nc = get_nc()
    in_maps, bias_total = make_in_maps(x, Wk, bk, Wv, bv, query, Wo, bo)
    res = run_bass_kernel_spmd(nc, in_maps, core_ids=list(range(NCORES)))
    out = np.concatenate([res.results[c]["out"] for c in range(NCORES)], axis=0)
    return (out + bias_total[None, :]).astype(np.float32)
